# revision 6
# baseline (speedup 1.0000x reference)
"""Trainium2 Bass kernel for a batched 2D Haar DWT (single level).

Input : x (8, 64, 512, 512) float32
Output: tuple (ll, lh, hl, hh), each (8, 64, 256, 256) float32, matching

    a00 = x[..., 0::2, 0::2]; a01 = x[..., 0::2, 1::2]
    a10 = x[..., 1::2, 0::2]; a11 = x[..., 1::2, 1::2]
    ll = (a00 + a01 + a10 + a11)/2
    lh = (a00 + a01 - a10 - a11)/2
    hl = (a00 - a01 + a10 - a11)/2
    hh = (a00 - a01 - a10 + a11)/2

Sharding: pure data parallel over the batch dim — core i processes x[i]
(64, 512, 512), no communication.

Per-core dataflow (per channel plane, 512x512 f32 = 1 MiB):
  - One contiguous 1 MiB DMA lands the plane as an SBUF tile [128, 2048]
    where partition p holds rows {2p, 2p+1} of the top half (free 0:1024)
    and rows {256+2p, 256+2p+1} of the bottom half (free 1024:2048).
  - ScalarE prescales the odd rows by 0.5 (frees the vector engines from
    the scale op).
  - VectorE scalar_tensor_tensor computes S = 0.5*even + 0.5*odd and
    D = 0.5*even - 0.5*odd (row butterfly).
  - Column butterfly: ll/hl from S on VectorE, lh/hh from D on GpSimd
    (stride-2 fp32 reads are 8-byte steps — at line rate on both).
  - Four 256 KiB DMAs store each subband plane.
"""

import sys

import numpy as np

for _p in ("/opt/trn_rl_repo",):
    if _p not in sys.path:
        sys.path.insert(0, _p)

from concourse import bacc, bass, mybir  # noqa: E402
from concourse.bass_utils import run_bass_kernel_spmd  # noqa: E402
from concourse.tile import TileContext  # noqa: E402

N_CORES = 8
C, H, W = 64, 512, 512
OUT_KEYS = ("ll", "lh", "hl", "hh")


def build_dwt(c_dim=C, h_dim=H, w_dim=W, bufs=3):
    """Build the per-core Bass module for a (c_dim, h_dim, w_dim) input."""
    f32 = mybir.dt.float32
    r_dim = h_dim // 2          # row pairs per channel
    p_dim = min(r_dim, 128)     # partitions used
    hblk = r_dim // p_dim       # partition-tiles per channel
    assert r_dim % p_dim == 0 and w_dim % 2 == 0
    wo = w_dim // 2

    nc = bacc.Bacc("TRN2", target_bir_lowering=False, debug=False)
    x = nc.dram_tensor("x", (c_dim, h_dim, w_dim), f32, kind="ExternalInput").ap()
    outs = {
        k: nc.dram_tensor(k, (c_dim, r_dim, wo), f32, kind="ExternalOutput").ap()
        for k in OUT_KEYS
    }
    add = mybir.AluOpType.add
    sub = mybir.AluOpType.subtract

    with TileContext(nc) as tc:
        with tc.tile_pool(name="pool", bufs=bufs) as pool:
            for c in range(c_dim):
                # --- load one channel plane, row-pairs on partitions ---
                xt = pool.tile([p_dim, hblk * 2 * w_dim], f32, tag="xt", name="xt")
                xv = xt.rearrange("p (h r w) -> p h r w", h=hblk, r=2, w=w_dim)
                src = x[c].rearrange("(h p r) w -> p h r w", h=hblk, p=p_dim, r=2)
                nc.sync.dma_start(out=xv, in_=src)
                # --- prescale by 1/2 on ScalarE (in place) ---
                # (scalar_tensor_tensor would fuse this, but TensorScalarPtr
                # instructions can't encode >1 sync wait — walrus rejects.)
                nc.scalar.mul(xt, xt, 0.5)
                ev = xv[:, :, 0]   # even rows * 0.5  [p, hblk, w]
                ov = xv[:, :, 1]   # odd rows * 0.5

                # --- row butterfly ---
                s = pool.tile([p_dim, hblk * w_dim], f32, tag="s", name="s")
                d = pool.tile([p_dim, hblk * w_dim], f32, tag="d", name="d")
                sv = s.rearrange("p (h w) -> p h w", h=hblk)
                dv = d.rearrange("p (h w) -> p h w", h=hblk)
                nc.vector.tensor_add(sv, ev, ov)
                nc.vector.tensor_sub(dv, ev, ov)

                # --- column butterfly ---
                se = s.rearrange("p (h l two) -> p h l two", h=hblk, two=2)
                de = d.rearrange("p (h l two) -> p h l two", h=hblk, two=2)
                ob = {
                    k: pool.tile([p_dim, hblk * wo], f32, tag=f"{k}b", name=f"{k}b")
                    for k in OUT_KEYS
                }
                obv = {
                    k: t.rearrange("p (h l) -> p h l", h=hblk)
                    for k, t in ob.items()
                }
                nc.vector.tensor_add(obv["ll"], se[:, :, :, 0], se[:, :, :, 1])
                nc.vector.tensor_sub(obv["hl"], se[:, :, :, 0], se[:, :, :, 1])
                nc.gpsimd.tensor_tensor(obv["lh"], de[:, :, :, 0], de[:, :, :, 1], add)
                nc.gpsimd.tensor_tensor(obv["hh"], de[:, :, :, 0], de[:, :, :, 1], sub)

                # --- store subband planes ---
                for k in OUT_KEYS:
                    dst = outs[k][c].rearrange("(h p) l -> p h l", h=hblk, p=p_dim)
                    nc.sync.dma_start(out=dst, in_=obv[k])
    nc.finalize()
    return nc


_CACHE = {}


def _get_nc():
    if "nc" not in _CACHE:
        _CACHE["nc"] = build_dwt()
    return _CACHE["nc"]


def kernel(x):
    x = np.ascontiguousarray(np.asarray(x), dtype=np.float32)
    assert x.shape == (N_CORES, C, H, W), x.shape
    nc = _get_nc()
    in_maps = [{"x": x[i]} for i in range(N_CORES)]
    res = run_bass_kernel_spmd(nc, in_maps, core_ids=list(range(N_CORES)))
    results = res.results
    return tuple(
        np.stack([np.asarray(results[i][k]) for i in range(N_CORES)], axis=0)
        for k in OUT_KEYS
    )


# revision 9
# speedup vs baseline: 1.0348x; 1.0348x over previous
"""Trainium2 Bass kernel for a batched 2D Haar DWT (single level).

Input : x (8, 64, 512, 512) float32
Output: tuple (ll, lh, hl, hh), each (8, 64, 256, 256) float32, matching

    a00 = x[..., 0::2, 0::2]; a01 = x[..., 0::2, 1::2]
    a10 = x[..., 1::2, 0::2]; a11 = x[..., 1::2, 1::2]
    ll = (a00 + a01 + a10 + a11)/2
    lh = (a00 + a01 - a10 - a11)/2
    hl = (a00 - a01 + a10 - a11)/2
    hh = (a00 - a01 - a10 + a11)/2

Sharding: pure data parallel over the batch dim — core i processes x[i]
(64, 512, 512), no communication.

Per-core dataflow (per channel plane, 512x512 f32 = 1 MiB):
  - One contiguous 1 MiB DMA lands the plane as an SBUF tile [128, 2048]
    where partition p holds rows {2p, 2p+1} of the top half (free 0:1024)
    and rows {256+2p, 256+2p+1} of the bottom half (free 1024:2048).
  - ScalarE prescales the odd rows by 0.5 (frees the vector engines from
    the scale op).
  - VectorE scalar_tensor_tensor computes S = 0.5*even + 0.5*odd and
    D = 0.5*even - 0.5*odd (row butterfly).
  - Column butterfly: ll/hl from S on VectorE, lh/hh from D on GpSimd
    (stride-2 fp32 reads are 8-byte steps — at line rate on both).
  - Four 256 KiB DMAs store each subband plane.
"""

import sys

import numpy as np

for _p in ("/opt/trn_rl_repo",):
    if _p not in sys.path:
        sys.path.insert(0, _p)

from concourse import bacc, bass, mybir  # noqa: E402
from concourse.bass_utils import run_bass_kernel_spmd  # noqa: E402
from concourse.tile import TileContext  # noqa: E402

N_CORES = 8
C, H, W = 64, 512, 512
OUT_KEYS = ("ll", "lh", "hl", "hh")


def build_dwt(c_dim=C, h_dim=H, w_dim=W, bufs=3):
    """Build the per-core Bass module for a (c_dim, h_dim, w_dim) input."""
    f32 = mybir.dt.float32
    r_dim = h_dim // 2          # row pairs per channel
    p_dim = min(r_dim, 128)     # partitions used
    hblk = r_dim // p_dim       # consecutive row-pairs per partition
    assert r_dim % p_dim == 0 and w_dim % 2 == 0
    wo = w_dim // 2

    nc = bacc.Bacc("TRN2", target_bir_lowering=False, debug=False)
    x = nc.dram_tensor("x", (c_dim, h_dim, w_dim), f32, kind="ExternalInput").ap()
    outs = {
        k: nc.dram_tensor(k, (c_dim, r_dim, wo), f32, kind="ExternalOutput").ap()
        for k in OUT_KEYS
    }
    add = mybir.AluOpType.add
    sub = mybir.AluOpType.subtract

    with TileContext(nc) as tc:
        with tc.tile_pool(name="pool", bufs=bufs) as pool:
            for c in range(c_dim):
                # --- load one channel plane, row-pairs on partitions ---
                # partition p holds input rows [2*hblk*p, 2*hblk*(p+1)) —
                # one contiguous 2*hblk*w*4-byte DMA chunk per partition,
                # and output rows land contiguously per partition too.
                xt = pool.tile([p_dim, hblk * 2 * w_dim], f32, tag="xt", name="xt")
                xv = xt.rearrange("p (h r w) -> p h r w", h=hblk, r=2, w=w_dim)
                src = x[c].rearrange("(p h r) w -> p h r w", h=hblk, p=p_dim, r=2)
                nc.sync.dma_start(out=xv, in_=src)
                # --- prescale by 1/2 on ScalarE (in place) ---
                # (scalar_tensor_tensor would fuse this, but TensorScalarPtr
                # instructions can't encode >1 sync wait — walrus rejects.)
                nc.scalar.mul(xt, xt, 0.5)
                ev = xv[:, :, 0]   # even rows * 0.5  [p, hblk, w]
                ov = xv[:, :, 1]   # odd rows * 0.5

                # --- row butterfly ---
                s = pool.tile([p_dim, hblk * w_dim], f32, tag="s", name="s")
                d = pool.tile([p_dim, hblk * w_dim], f32, tag="d", name="d")
                sv = s.rearrange("p (h w) -> p h w", h=hblk)
                dv = d.rearrange("p (h w) -> p h w", h=hblk)
                nc.vector.tensor_add(sv, ev, ov)
                nc.vector.tensor_sub(dv, ev, ov)

                # --- column butterfly ---
                se = s.rearrange("p (h l two) -> p h l two", h=hblk, two=2)
                de = d.rearrange("p (h l two) -> p h l two", h=hblk, two=2)
                ob = {
                    k: pool.tile([p_dim, hblk * wo], f32, tag=f"{k}b", name=f"{k}b")
                    for k in OUT_KEYS
                }
                obv = {
                    k: t.rearrange("p (h l) -> p h l", h=hblk)
                    for k, t in ob.items()
                }
                nc.vector.tensor_add(obv["ll"], se[:, :, :, 0], se[:, :, :, 1])
                nc.vector.tensor_sub(obv["hl"], se[:, :, :, 0], se[:, :, :, 1])
                nc.gpsimd.tensor_tensor(obv["lh"], de[:, :, :, 0], de[:, :, :, 1], add)
                nc.gpsimd.tensor_tensor(obv["hh"], de[:, :, :, 0], de[:, :, :, 1], sub)

                # --- store subband planes ---
                for k in OUT_KEYS:
                    dst = outs[k][c].rearrange("(p h) l -> p h l", h=hblk, p=p_dim)
                    # second HWDGE ring (ACT sequencer) for stores
                    nc.scalar.dma_start(out=dst, in_=obv[k])
    nc.finalize()
    return nc


_CACHE = {}


def _get_nc():
    if "nc" not in _CACHE:
        _CACHE["nc"] = build_dwt()
    return _CACHE["nc"]


def kernel(x):
    x = np.ascontiguousarray(np.asarray(x), dtype=np.float32)
    assert x.shape == (N_CORES, C, H, W), x.shape
    nc = _get_nc()
    in_maps = [{"x": x[i]} for i in range(N_CORES)]
    res = run_bass_kernel_spmd(nc, in_maps, core_ids=list(range(N_CORES)))
    results = res.results
    return tuple(
        np.stack([np.asarray(results[i][k]) for i in range(N_CORES)], axis=0)
        for k in OUT_KEYS
    )
